# revision 1
# baseline (speedup 1.0000x reference)
"""AttnBlock (GroupNorm + single-head 1x1-conv attention + residual) on 8 TRN2 cores.

Sharding: core c handles batch b = c//2, query-token half c%2 (2048 of 4096
tokens). Each core computes GN + attention for its query half against all keys
of its batch element, returning [512, 2048]; host reassembles [4,512,64,64].

Algebraic folds (exact):
  - S = scale*(wq h + bq)@(wk h + bk):  the bk term is constant along the
    softmax axis and cancels; the rest is h^T @ (Mt h + m) with
    Mt = scale*wk^T wq, m = scale*wk^T bq  -> K and Q never materialize.
  - v-bias bv contributes wp@bv to every output token -> folded into bp.

All heavy matmuls run as float32r (near-bf16 PE rate, ~1.5e-4 rel err per
K=128 contraction). Softmax runs unnormalized (exp without max-subtraction is
safe at these logit scales); the 1/L scale is deferred past the linear proj
to the final elementwise stage. The j-loop emission is software-pipelined
(O-matmuls trail the S^T matmuls by DEPTH chunks) because the in-order PE
stream otherwise stalls at each exp dependency.
"""

import numpy as np

B, C, HW = 4, 512, 64
N = HW * HW            # 4096 tokens
NQ = N // 2            # 2048 query tokens per core
NT = C // 128          # 4 channel tiles
NJ = N // 128          # 32 key chunks
NBLK = NQ // 512       # 4 query blocks of 512
NG = 32                # groups
EPS = 1e-6
SCALE = 1.0 / np.sqrt(C)

_CACHE = {}


def _build_nc(reps=1):
    import contextlib
    import concourse.bass as bass
    import concourse.mybir as mybir
    import concourse.tile as tile
    import concourse.bacc as bacc

    f32 = mybir.dt.float32
    f32r = mybir.dt.float32r
    AF = mybir.ActivationFunctionType
    OP = mybir.AluOpType

    nc = bacc.Bacc("TRN2", target_bir_lowering=False, debug=False, num_devices=8)

    x_d = nc.dram_tensor("x", [C, N], f32r, kind="ExternalInput").ap()
    mt_d = nc.dram_tensor("mt", [C, C], f32r, kind="ExternalInput").ap()
    wvT_d = nc.dram_tensor("wvt", [C, C], f32r, kind="ExternalInput").ap()
    wpT_d = nc.dram_tensor("wpt", [C, C], f32r, kind="ExternalInput").ap()
    mvec_d = nc.dram_tensor("mvec", [128, NT], f32, kind="ExternalInput").ap()
    bpp_d = nc.dram_tensor("bpp", [128, NT], f32, kind="ExternalInput").ap()
    gnsc_d = nc.dram_tensor("gnsc", [128, NT], f32, kind="ExternalInput").ap()
    gnbi_d = nc.dram_tensor("gnbi", [128, NT], f32, kind="ExternalInput").ap()
    ind_d = nc.dram_tensor("ind", [C, NG], f32, kind="ExternalInput").ap()
    emat_d = nc.dram_tensor("emat", [NG, 128], f32, kind="ExternalInput").ap()
    tmask_d = nc.dram_tensor("tmask", [NG, NT], f32, kind="ExternalInput").ap()
    ones_d = nc.dram_tensor("ones", [128, 1], f32r, kind="ExternalInput").ap()
    y_d = nc.dram_tensor("y", [C, NQ], f32, kind="ExternalOutput").ap()

    # Each core gets its own x with its query half rolled to tokens
    # [0, 2048); key-side sums run over all N tokens so the roll is exact.

    x_t = x_d.rearrange("(t p) n -> t p n", p=128)
    y_t = y_d.rearrange("(t p) n -> t p n", p=128)

    with tile.TileContext(nc) as tc:
        with (
            tc.tile_pool(name="hpool", bufs=1) as hpool,
            tc.tile_pool(name="vpool", bufs=1) as vpool,
            tc.tile_pool(name="wpool", bufs=1) as wpool,
            tc.tile_pool(name="cpool", bufs=1) as cpool,
            tc.tile_pool(name="gn", bufs=1) as gn,
            tc.tile_pool(name="qt", bufs=2) as qtp,
            tc.tile_pool(name="pt", bufs=6) as ptp,
            tc.tile_pool(name="fin", bufs=1) as finp,
            tc.tile_pool(name="ps512", bufs=4, space="PSUM") as ps512,
            tc.tile_pool(name="psO", bufs=1, space="PSUM") as psO,
        ):
            with (tc.For_i(0, reps, 1) if reps > 1 else contextlib.nullcontext()):
                # ---- load x (becomes h in place after GN) ----
                h = []
                for t in range(NT):
                    ht = hpool.tile([128, N], f32r, name=f"h{t}", tag=f"h{t}")
                    for s in range(8):
                        nc.sync.dma_start(ht[:, s * 512:(s + 1) * 512],
                                          x_t[t][:, s * 512:(s + 1) * 512])
                    h.append(ht)

                # ---- load weights/constants ----
                mt_sb = wpool.tile([128, NT, C], f32r)
                wvT_sb = wpool.tile([128, NT, C], f32r)
                wpT_sb = wpool.tile([128, NT, C], f32r)
                nc.sync.dma_start(mt_sb[:], mt_d.rearrange("(t p) n -> p t n", p=128))
                nc.sync.dma_start(wvT_sb[:], wvT_d.rearrange("(t p) n -> p t n", p=128))
                nc.sync.dma_start(wpT_sb[:], wpT_d.rearrange("(t p) n -> p t n", p=128))
                mvec_sb = cpool.tile([128, NT], f32)
                bpp_sb = cpool.tile([128, NT], f32)
                gnsc_sb = cpool.tile([128, NT], f32)
                gnbi_sb = cpool.tile([128, NT], f32)
                ind_sb = cpool.tile([128, NT, NG], f32)
                emat_sb = cpool.tile([NG, 128], f32)
                tmask_sb = cpool.tile([NG, NT], f32)
                ones_sb = cpool.tile([128, 1], f32r)
                nc.sync.dma_start(mvec_sb[:], mvec_d[:])
                nc.sync.dma_start(bpp_sb[:], bpp_d[:])
                nc.sync.dma_start(gnsc_sb[:], gnsc_d[:])
                nc.sync.dma_start(gnbi_sb[:], gnbi_d[:])
                nc.sync.dma_start(ind_sb[:], ind_d.rearrange("(t p) g -> p t g", p=128))
                nc.sync.dma_start(emat_sb[:], emat_d[:])
                nc.sync.dma_start(tmask_sb[:], tmask_d[:])
                nc.sync.dma_start(ones_sb[:], ones_d[:])

                # ---- GroupNorm stats ----
                # per-channel mean/var via bn_stats (512-wide chunks) + bn_aggr
                stats3 = []
                for t in range(NT):
                    bnb = gn.tile([128, 8, 6], f32, name=f"bnb{t}", tag="bnb", bufs=2)
                    for s in range(8):
                        nc.vector.bn_stats(
                            bnb[:, s, :], h[t][:].bitcast(f32)[:, s * 512:(s + 1) * 512]
                        )
                    mv = gn.tile([128, 2], f32, name=f"mv{t}", tag="mv", bufs=2)
                    nc.vector.bn_aggr(mv[:], bnb[:])
                    s3 = gn.tile([128, 3], f32, name=f"s3_{t}", tag=f"s3_{t}")
                    nc.vector.tensor_copy(s3[:, 0:2], mv[:])
                    nc.scalar.square(s3[:, 2:3], mv[:, 0:1])
                    stats3.append(s3)
                # group aggregation: [32, 3] = sum_c ind[c, g] * [mean, var, mean^2]
                ps_g = ps512.tile([NG, 3], f32, tag="ps512")
                for t in range(NT):
                    nc.tensor.matmul(ps_g[:], ind_sb[:, t, :], stats3[t][:],
                                     start=(t == 0), stop=(t == NT - 1))
                # var_g = avg_var + avg_mean2 - avg_mean^2 ; inv = 1/sqrt(var+eps)
                sg = gn.tile([NG, 3], f32)
                nc.vector.tensor_copy(sg[:], ps_g[:])
                msq = gn.tile([NG, 1], f32)
                nc.scalar.square(msq[:], sg[:, 0:1])
                vg = gn.tile([NG, 1], f32)
                nc.vector.tensor_add(vg[:], sg[:, 1:2], sg[:, 2:3])
                nc.vector.tensor_sub(vg[:], vg[:], msq[:])
                eps_t = gn.tile([NG, 1], f32)
                nc.vector.memset(eps_t[:], EPS)
                std = gn.tile([NG, 1], f32)
                nc.scalar.activation(std[:], vg[:], AF.Sqrt, bias=eps_t[:])
                inv = gn.tile([NG, 1], f32)
                nc.vector.reciprocal(inv[:], std[:])
                mcol = sg[:, 0:1]
                # spread group values back to channel layout via E-matmul
                rmat = gn.tile([NG, 2 * NT], f32)
                nc.vector.tensor_scalar_mul(rmat[:, 0:NT], tmask_sb[:], inv[:])
                nc.vector.tensor_scalar_mul(rmat[:, NT:2 * NT], tmask_sb[:], mcol[:])
                ps_e = ps512.tile([128, 2 * NT], f32, tag="ps512")
                nc.tensor.matmul(ps_e[:], emat_sb[:], rmat[:], start=True, stop=True)
                a_pc = gn.tile([128, NT], f32)
                b_pc = gn.tile([128, NT], f32)
                nc.vector.tensor_mul(a_pc[:], gnsc_sb[:], ps_e[:, 0:NT])
                nc.vector.tensor_mul(b_pc[:], ps_e[:, NT:2 * NT], a_pc[:])
                nc.vector.tensor_sub(b_pc[:], gnbi_sb[:], b_pc[:])
                # apply: h = a*x + b  (in place, DVE, 512-col chunks so the
                # first V^T / q-tilde matmuls start as soon as their columns
                # are normalized rather than after all 16K columns)
                for s in range(8):
                    for t in range(NT):
                        nc.vector.tensor_scalar(
                            h[t][:, s * 512:(s + 1) * 512],
                            h[t][:].bitcast(f32)[:, s * 512:(s + 1) * 512],
                            a_pc[:, t:t + 1], b_pc[:, t:t + 1],
                            OP.mult, OP.add)

                # ---- V^T = (wv h)^T : [N, C] as 32 chunks [128, 512] ----
                vT = []
                for js in range(NJ):
                    ps_v = ps512.tile([128, 512], f32, tag="ps512")
                    for t in range(NT):
                        nc.tensor.matmul(ps_v[:], h[t][:, js * 128:(js + 1) * 128],
                                         wvT_sb[:, t, :],
                                         start=(t == 0), stop=(t == NT - 1))
                    vt = vpool.tile([128, 512], f32r, name=f"vt{js}", tag=f"vt{js}")
                    if js % 2 == 0:
                        nc.scalar.copy(vt[:], ps_v[:])
                    else:
                        nc.vector.tensor_copy(vt[:], ps_v[:])
                    vT.append(vt)

                # ---- attention over 4 query blocks of 512 ----
                def emit_qtil(ib):
                    isel = slice(ib * 512, (ib + 1) * 512)
                    qtil = qtp.tile([128, NT, 512], f32r, name=f"qtil{ib}", tag="qt")
                    for co in range(NT):
                        ps_q = ps512.tile([128, 512], f32, tag="ps512")
                        for t in range(NT):
                            nc.tensor.matmul(
                                ps_q[:], mt_sb[:, t, co * 128:(co + 1) * 128],
                                h[t][:, isel], start=(t == 0), stop=(t == NT - 1))
                        nc.vector.tensor_scalar_add(qtil[:, co, :], ps_q[:],
                                                    mvec_sb[:, co:co + 1])
                    return qtil

                next_qtil = emit_qtil(0)
                for ib in range(NBLK):
                    isel = slice(ib * 512, (ib + 1) * 512)
                    qtil = next_qtil
                    # j-loop: S^T chunk -> exp -> O/L accumulation
                    ps_o = [psO.tile([128, 512], f32, name=f"o{ib}_{cs}", tag=f"psO{cs}")
                            for cs in range(NT)]
                    lacc = finp.tile([128, 512], f32, tag="lacc")
                    # software pipeline: O-matmuls for chunk js are emitted
                    # after S^T of chunk js+2, hiding the exp latency from
                    # the in-order PE stream.
                    DEPTH = 4
                    pend = []

                    def emit_o(js, pT):
                        for cs in range(NT):
                            nc.tensor.matmul(ps_o[cs][:],
                                             vT[js][:, cs * 128:(cs + 1) * 128],
                                             pT[:],
                                             start=(js == 0), stop=(js == NJ - 1))

                    for js in range(NJ):
                        ps_s = ps512.tile([128, 512], f32, tag="ps512")
                        for t in range(NT):
                            nc.tensor.matmul(ps_s[:], h[t][:, js * 128:(js + 1) * 128],
                                             qtil[:, t, :],
                                             start=(t == 0), stop=(t == NT - 1))
                        pT = ptp.tile([128, 512], f32r, tag="pt")
                        nc.scalar.activation(pT[:], ps_s[:], AF.Exp)
                        if js == 0:
                            nc.vector.tensor_copy(lacc[:], pT[:].bitcast(f32))
                        else:
                            nc.vector.tensor_add(lacc[:], lacc[:], pT[:].bitcast(f32))
                        pend.append((js, pT))
                        if len(pend) > DEPTH:
                            emit_o(*pend.pop(0))
                    for item in pend:
                        emit_o(*item)
                    # drain O unnormalized (proj is linear; 1/L applied at the end)
                    o_sb = qtp.tile([128, NT, 512], f32r, name="o_sb", tag="qt")
                    for cs in range(NT):
                        nc.vector.tensor_copy(o_sb[:, cs, :], ps_o[cs][:])
                    if ib + 1 < NBLK:
                        next_qtil = emit_qtil(ib + 1)
                    ps_l = ps512.tile([1, 512], f32, tag="ps512")
                    nc.tensor.matmul(ps_l[:], ones_sb[:].bitcast(f32), lacc[:],
                                     start=True, stop=True)
                    lrec = finp.tile([1, 512], f32, tag="lrec")
                    nc.vector.reciprocal(lrec[:], ps_l[:])
                    lb = finp.tile([128, 512], f32, tag="lb")
                    nc.gpsimd.partition_broadcast(lb[:], lrec[:])
                    # proj + bias + residual
                    res = finp.tile([128, NT, 512], f32r, tag="res")
                    for t in range(NT):
                        nc.sync.dma_start(res[:, t, :], x_t[t][:, isel])
                    out_sb = finp.tile([128, NT, 512], f32, tag="out")
                    for os_ in range(NT):
                        ps_p = ps512.tile([128, 512], f32, tag="ps512")
                        for cs in range(NT):
                            nc.tensor.matmul(
                                ps_p[:], wpT_sb[:, cs, os_ * 128:(os_ + 1) * 128],
                                o_sb[:, cs, :], start=(cs == 0), stop=(cs == NT - 1))
                        nc.vector.scalar_tensor_tensor(
                            out_sb[:, os_, :], ps_p[:], 1.0, lb[:],
                            OP.mult, OP.mult)
                        nc.vector.scalar_tensor_tensor(
                            out_sb[:, os_, :], out_sb[:, os_, :],
                            bpp_sb[:, os_:os_ + 1], res[:, os_, :].bitcast(f32),
                            OP.add, OP.add)
                    nc.sync.dma_start(y_t[:, :, isel].rearrange("t p n -> p t n"),
                                      out_sb[:])
    nc.compile()
    return nc


def _host_prep(gn_scale, gn_bias, wq, bq, wk, bk, wv, bv, wp, bp):
    f = np.float32

    def pc(v):  # [512] -> [128, 4] channel layout (c = t*128 + p)
        return np.ascontiguousarray(v.reshape(NT, 128).T).astype(f)

    wq64, wk64, wv64, wp64 = (np.asarray(w, np.float64) for w in (wq, wk, wv, wp))
    mt = (SCALE * (wq64.T @ wk64)).astype(f)                      # [c_in, c_out]
    mvec = pc((SCALE * (wk64.T @ np.asarray(bq, np.float64))).astype(f))
    bpp = pc((np.asarray(bp, np.float64) + wp64 @ np.asarray(bv, np.float64)).astype(f))
    wvT = np.ascontiguousarray(wv64.T).astype(f)
    wpT = np.ascontiguousarray(wp64.T).astype(f)

    ind = np.zeros((C, NG), f)
    ind[np.arange(C), np.arange(C) // 16] = 1.0 / 16.0
    emat = np.zeros((NG, 128), f)
    for g in range(NG):
        for p in range(128):
            if p // 16 == g % 8:
                emat[g, p] = 1.0
    tmask = np.zeros((NG, NT), f)
    for g in range(NG):
        tmask[g, g // 8] = 1.0
    ones = np.ones((128, 1), f)

    return dict(
        mt=mt, wvt=wvT, wpt=wpT, mvec=mvec, bpp=bpp,
        gnsc=pc(np.asarray(gn_scale, f)), gnbi=pc(np.asarray(gn_bias, f)),
        ind=ind, emat=emat, tmask=tmask, ones=ones,
    )


def kernel(hidden_states, gn_scale, gn_bias, wq, bq, wk, bk, wv, bv, wp, bp):
    from concourse.bass_utils import run_bass_kernel_spmd

    if "nc" not in _CACHE:
        _CACHE["nc"] = _build_nc()
    nc = _CACHE["nc"]

    shared = _host_prep(gn_scale, gn_bias, wq, bq, wk, bk, wv, bv, wp, bp)
    x = np.asarray(hidden_states, np.float32).reshape(B, C, N)

    in_maps = []
    for c in range(8):
        b, half = c // 2, c % 2
        xb = x[b]
        if half:
            # roll so this core's query tokens sit at [0, 2048)
            xb = np.concatenate([xb[:, NQ:], xb[:, :NQ]], axis=1)
        m = dict(shared)
        m["x"] = np.ascontiguousarray(xb)
        in_maps.append(m)

    res = run_bass_kernel_spmd(nc, in_maps, list(range(8)))

    out = np.empty((B, C, N), np.float32)
    for c in range(8):
        b, half = c // 2, c % 2
        out[b][:, half * NQ:(half + 1) * NQ] = res.results[c]["y"]
    return out.reshape(B, C, HW, HW)

